# revision 6
# baseline (speedup 1.0000x reference)
"""Trainium2 Bass kernel for nn_Attention_66907000537586.

Module: x -> 1x1conv+BN (Q,K,V) -> 8-head attention with relative position
bias -> GELU -> 1x1conv+bias+BN.  Shapes: B=8, C=256, F=32 (n=1024 tokens),
H=8, DK=32, DV=64.

Sharding: pure data-parallel over batch (one batch element per NeuronCore,
8 cores), no collectives.  All BN/scale folding happens on host; the
relative-position bias is compressed host-side into its 63 distinct
32x32 Toeplitz blocks per head and injected into the dots PSUM on device
via identity matmuls (accumulating on top of the Q.K^T product).

Device dataflow per core (batch element):
  X (256,1024) --PE--> Q,K (256,1024 fp32, BN folded, scale folded into Q)
                 --PE--> V^T (1024, 8*(64+1) interleaved, bf16; per head 64 V
                         columns + a ones column that makes the attention
                         matmul also produce the softmax denominator)
  per head h, per j-tile (128 rows of K):
     dots^T[j,i] = K_h^T Q_h via matmul (K=32 contraction)  [PSUM fp32]
     + bias^T via identity-matmul copy from the compact bias table [accum]
     exp() on ScalarE PSUM->SBUF bf16 (softmax without max-subtraction --
       dots+bias is in [-8, 8], exp is safe in fp32/bf16)
     OU^T[d,i] += V^T_h(j-tile)^T @ exp-tile   [accumulated over j-tiles]
  OU rows 0..63 = unnormalized output, row 64 = sum_j exp = denominator S
  recip(S) on VectorE, broadcast across partitions via SBUF->SBUF DMA,
  normalize on VectorE, exact GELU on ScalarE -> G (512, 1024) fp32
  Y = Wo_folded @ G + bias per-partition  -> out (256, 1024) fp32
"""

import numpy as np
import ml_dtypes

HEADS, DK, DV, F = 8, 32, 64, 32
C = 256
N = F * F            # 1024 tokens
B = 8
EPS = 1e-5
IDK = HEADS * DK     # 256
IDV = HEADS * DV     # 512
VTW = HEADS * (DV + 1)   # 520: per head 64 V cols + 1 ones col (ones LAST)
SW = 2112            # per-head width of the shifted compact bias table
NJT = N // 128       # 8 j-tiles
NIT = N // 512       # 2 i-tiles

_PROGRAM_CACHE = {}


def _split_excess_waits(nc, mybir, limit=1):
    """The walrus build in this container rejects instructions carrying more
    than `limit` semaphore sync-waits.  Move the excess onto carrier NoOps
    inserted just before, on the same engine (same queue => same ordering)."""
    k = 0
    for fn in nc.m.functions:
        for bb in fn.blocks:
            out = []
            for inst in bb.instructions:
                si = inst.sync_info
                waits = list(si.on_wait) if si is not None else []
                if len(waits) > limit:
                    extra, keep = waits[:-limit], waits[-limit:]
                    for i in range(0, len(extra), limit):
                        nop = mybir.InstNoOp(name=f"waitsplit_{k}")
                        k += 1
                        nop.engine = inst.engine
                        nop.sync_info = mybir.SyncInfo(
                            on_wait=extra[i:i + limit], on_update=[])
                        out.append(nop)
                    si.on_wait = keep
                out.append(inst)
            bb.instructions = out


def build_program(structured=True):
    """Build the single-core Bass program (run SPMD on 8 cores)."""
    import concourse.bass as bass
    import concourse.mybir as mybir
    import concourse.tile as tile

    dt = mybir.dt
    PTC = tile.TileContext

    nc = bass.Bass("TRN2", target_bir_lowering=False, debug=False, num_devices=B)

    x = nc.dram_tensor("x", [C, N], dt.float32, kind="ExternalInput")
    wqt = nc.dram_tensor("wqt", [C, IDK], dt.float32, kind="ExternalInput")
    wkt = nc.dram_tensor("wkt", [C, IDK], dt.float32, kind="ExternalInput")
    wvt = nc.dram_tensor("wvt", [C, VTW], dt.float32, kind="ExternalInput")
    qkoff = nc.dram_tensor("qkoff", [128, 4], dt.float32, kind="ExternalInput")
    voffi = nc.dram_tensor("voffi", [1, VTW], dt.float32, kind="ExternalInput")
    wot = nc.dram_tensor("wot", [IDV, C], dt.float32, kind="ExternalInput")
    ooff = nc.dram_tensor("ooff", [128, 2], dt.float32, kind="ExternalInput")
    if structured:
        sst = nc.dram_tensor("sst", [128, HEADS * SW], dt.bfloat16, kind="ExternalInput")
    else:
        sst = nc.dram_tensor("sst", [HEADS * NJT * 128, N], dt.bfloat16, kind="ExternalInput")
    out = nc.dram_tensor("out", [C, N], dt.float32, kind="ExternalOutput")

    ident_np = np.eye(128, dtype=ml_dtypes.bfloat16)
    ident_dram = nc.inline_tensor(ident_np, name="ident128")

    with PTC(nc) as tc:
        with (
            tc.tile_pool(name="persist", bufs=1) as pp,
            tc.tile_pool(name="exps", bufs=3) as ep,
            tc.tile_pool(name="norm", bufs=2) as np_pool,
            tc.tile_pool(name="bias_stream", bufs=3) as bp,
            tc.tile_pool(name="dramscratch", bufs=2, space="DRAM") as dp,
            tc.tile_pool(name="psA", bufs=2, space="PSUM") as psA,
            tc.tile_pool(name="psB", bufs=2, space="PSUM") as psB,
        ):
            f32 = dt.float32
            bf16 = dt.bfloat16

            # ---- persistent SBUF tiles + input DMAs ----
            xsb = [pp.tile([128, N], f32, tag=f"xsb{k}", name=f"xsb{k}") for k in range(2)]
            for k in range(2):
                nc.sync.dma_start(out=xsb[k], in_=x.ap()[k * 128:(k + 1) * 128, :])
            wqtsb = [pp.tile([128, IDK], f32, tag=f"wqt{k}", name=f"wqt{k}") for k in range(2)]
            wktsb = [pp.tile([128, IDK], f32, tag=f"wkt{k}", name=f"wkt{k}") for k in range(2)]
            wvtsb = [pp.tile([128, VTW], f32, tag=f"wvt{k}", name=f"wvt{k}") for k in range(2)]
            for k in range(2):
                nc.sync.dma_start(out=wqtsb[k], in_=wqt.ap()[k * 128:(k + 1) * 128, :])
                nc.sync.dma_start(out=wktsb[k], in_=wkt.ap()[k * 128:(k + 1) * 128, :])
                nc.sync.dma_start(out=wvtsb[k], in_=wvt.ap()[k * 128:(k + 1) * 128, :])
            qkoffsb = pp.tile([128, 4], f32, tag="qkoff")
            nc.sync.dma_start(out=qkoffsb, in_=qkoff.ap())
            voffisb = pp.tile([1, VTW], f32, tag="voffi")
            nc.sync.dma_start(out=voffisb, in_=voffi.ap())
            wotsb = [pp.tile([128, C], f32, tag=f"wot{k}", name=f"wot{k}") for k in range(4)]
            for k in range(4):
                nc.sync.dma_start(out=wotsb[k], in_=wot.ap()[k * 128:(k + 1) * 128, :])
            ooffsb = pp.tile([128, 2], f32, tag="ooff")
            nc.sync.dma_start(out=ooffsb, in_=ooff.ap())
            identsb = pp.tile([128, 128], bf16, tag="ident")
            nc.sync.dma_start(out=identsb, in_=ident_dram.ap())
            if structured:
                sstsb = pp.tile([128, HEADS * SW], bf16, tag="sst")
                for h in range(HEADS):  # split DMA so head 0 arrives first
                    nc.sync.dma_start(
                        out=sstsb[:, h * SW:(h + 1) * SW],
                        in_=sst.ap()[:, h * SW:(h + 1) * SW],
                    )
            onesrow = pp.tile([1, N], f32, tag="onesrow")
            nc.vector.memset(onesrow, 1.0)

            # ---- projections ----
            qsb = [pp.tile([128, N], f32, tag=f"qsb{m}", name=f"qsb{m}") for m in range(2)]
            ksb = [pp.tile([128, N], f32, tag=f"ksb{m}", name=f"ksb{m}") for m in range(2)]
            vtsb = [pp.tile([128, VTW], bf16, tag=f"vtsb{j}", name=f"vtsb{j}") for j in range(NJT)]

            for m in range(2):  # Q then K, channel tile m
                for (wsb, dst, col) in ((wqtsb, qsb, m), (wktsb, ksb, 2 + m)):
                    ps = psA.tile([128, N], f32, tag="pA")
                    for nt in range(NIT):
                        for k in range(2):
                            nc.tensor.matmul(
                                ps[:, nt * 512:(nt + 1) * 512],
                                lhsT=wsb[k][:, m * 128:(m + 1) * 128],
                                rhs=xsb[k][:, nt * 512:(nt + 1) * 512],
                                start=(k == 0), stop=(k == 1),
                            )
                    nc.vector.tensor_scalar_add(dst[m], ps, qkoffsb[:, col:col + 1])

            for j in range(NJT):  # V^T j-tile
                ps = psA.tile([128, VTW], f32, tag="pA")
                for (lo, hi) in ((0, 512), (512, VTW)):
                    for k in range(2):
                        nc.tensor.matmul(
                            ps[:, lo:hi],
                            lhsT=xsb[k][:, j * 128:(j + 1) * 128],
                            rhs=wvtsb[k][:, lo:hi],
                            start=(k == 0), stop=False,
                        )
                    nc.tensor.matmul(  # + ones row x voffi  (BN offset + ones col)
                        ps[:, lo:hi],
                        lhsT=onesrow[0:1, j * 128:(j + 1) * 128],
                        rhs=voffisb[0:1, lo:hi],
                        start=False, stop=True,
                    )
                nc.vector.tensor_copy(vtsb[j], ps)  # fp32 -> bf16 cast

            # ---- attention per head ----
            gsb = [pp.tile([128, N], f32, tag=f"gsb{t}", name=f"gsb{t}") for t in range(4)]

            for h in range(HEADS):
                qt, rp = h // 4, 32 * (h % 4)
                ou = psB.tile([65, N], f32, tag="ou")
                for j in range(NJT):
                    dots = psA.tile([128, N], f32, tag="pA")
                    for it in range(NIT):
                        sl = slice(it * 512, (it + 1) * 512)
                        nc.tensor.matmul(
                            dots[:, sl],
                            lhsT=ksb[qt][rp:rp + 32, j * 128:(j + 1) * 128],
                            rhs=qsb[qt][rp:rp + 32, sl],
                            start=True, stop=False,
                            tile_position=(rp, 0),
                        )
                        if structured:
                            off = h * SW + (31 - 4 * j) * 32 + it * 512
                            rhs_bias = sstsb[:, off:off + 512]
                        else:
                            if it == 0:
                                btile = bp.tile([128, N], bf16, tag="bt")
                                base = (h * NJT + j) * 128
                                nc.sync.dma_start(
                                    out=btile, in_=sst.ap()[base:base + 128, :])
                            rhs_bias = btile[:, sl]
                        nc.tensor.matmul(
                            dots[:, sl], lhsT=identsb, rhs=rhs_bias,
                            start=False, stop=True,
                        )
                    et = ep.tile([128, N], bf16, tag="et")
                    nc.scalar.activation(et, dots, mybir.ActivationFunctionType.Exp)
                    for it in range(NIT):
                        sl = slice(it * 512, (it + 1) * 512)
                        nc.tensor.matmul(
                            ou[:, sl],
                            lhsT=vtsb[j][:, h * 65:h * 65 + 65],
                            rhs=et[:, sl],
                            start=(j == 0), stop=(j == NJT - 1),
                        )
                recip = np_pool.tile([1, N], f32, tag="recip")
                nc.vector.reciprocal(recip, ou[64:65, :])
                rd = dp.tile([1, N], f32, tag="rd", name="rd")
                nc.sync.dma_start(out=rd, in_=recip)
                rb = np_pool.tile([64, N], f32, tag="rb")
                nc.sync.dma_start(
                    out=rb,
                    in_=bass.AP(tensor=rd.tensor, offset=rd.offset, ap=[[0, 64], [1, N]]),
                )
                gn = np_pool.tile([64, N], f32, tag="gn")
                nc.vector.tensor_mul(gn, ou[0:64, :], rb)
                nc.scalar.activation(
                    gsb[h // 2][64 * (h % 2):64 * (h % 2) + 64, :],
                    gn, mybir.ActivationFunctionType.Gelu,
                )

            # ---- output conv ----
            for m in range(2):
                ps = psA.tile([128, N], f32, tag="pA")
                for nt in range(NIT):
                    sl = slice(nt * 512, (nt + 1) * 512)
                    for k in range(4):
                        nc.tensor.matmul(
                            ps[:, sl],
                            lhsT=wotsb[k][:, m * 128:(m + 1) * 128],
                            rhs=gsb[k][:, sl],
                            start=(k == 0), stop=(k == 3),
                        )
                ysb = np_pool.tile([128, N], f32, tag="ysb")
                nc.vector.tensor_scalar_add(ysb, ps, ooffsb[:, m:m + 1])
                nc.sync.dma_start(out=out.ap()[m * 128:(m + 1) * 128, :], in_=ysb)

    _split_excess_waits(nc, mybir)
    return nc


def _fold_inputs(inp):
    """Host-side BN/scale folding + compact bias table construction."""
    f32 = np.float32
    scale = DK ** -0.5
    x = np.asarray(inp["x"], f32)

    def bn_fold(w, gam, bet, mu, var, s=1.0):
        inv = np.asarray(gam, f32) / np.sqrt(np.asarray(var, f32) + EPS)
        wf = (np.asarray(w, f32) * inv[:, None] * s).T.copy()       # (C, O)
        off = (np.asarray(bet, f32) - np.asarray(mu, f32) * inv) * s  # (O,)
        return wf, off

    wqt, qoff = bn_fold(inp["wq"], inp["qgam"], inp["qbet"], inp["qmu"], inp["qvar"], scale)
    wkt, koff = bn_fold(inp["wk"], inp["kgam"], inp["kbet"], inp["kmu"], inp["kvar"])
    wvt0, voff = bn_fold(inp["wv"], inp["vgam"], inp["vbet"], inp["vmu"], inp["vvar"])

    wvt = np.zeros((C, VTW), f32)
    voffi = np.zeros((1, VTW), f32)
    for h in range(HEADS):
        wvt[:, 65 * h:65 * h + 64] = wvt0[:, 64 * h:64 * h + 64]
        voffi[0, 65 * h:65 * h + 64] = voff[64 * h:64 * h + 64]
        voffi[0, 65 * h + 64] = 1.0   # ones column -> softmax denominator

    oinv = np.asarray(inp["ogam"], f32) / np.sqrt(np.asarray(inp["ovar"], f32) + EPS)
    wot = (np.asarray(inp["wo"], f32) * oinv[:, None]).T.copy()      # (512, 256)
    ooff_v = (np.asarray(inp["bo"], f32) * oinv
              + np.asarray(inp["obet"], f32) - np.asarray(inp["omu"], f32) * oinv)

    qkoff = np.stack([qoff[:128], qoff[128:], koff[:128], koff[128:]], axis=1).copy()
    ooff = np.stack([ooff_v[:128], ooff_v[128:]], axis=1).copy()

    pe = np.asarray(inp["pos_emb"], f32)             # (1024, 8)
    pidx = np.asarray(inp["pos_indices"])            # (1024, 1024) int32

    r = np.arange(F)
    pos = np.stack(np.meshgrid(r, r, indexing="ij"), axis=-1).reshape(-1, 2)
    rel = np.abs(pos[:, None, :] - pos[None, :, :])
    expected = (rel[..., 0] * F + rel[..., 1]).astype(pidx.dtype)
    structured = bool(np.array_equal(pidx, expected))

    if structured:
        dd = np.arange(63)
        xi_ = np.maximum(dd - 31, 0)
        xj_ = np.maximum(31 - dd, 0)
        yy = np.arange(F)
        I = xi_[:, None] * F + yy[None, :]           # (63, yi)
        J = xj_[:, None] * F + yy[None, :]           # (63, yj)
        idx = pidx[I[:, None, :], J[:, :, None]]     # (dd, yj, yi)
        sb = pe[idx] / scale                         # (dd, yj, yi, H)
        flat = np.ascontiguousarray(sb.transpose(3, 1, 0, 2)).reshape(HEADS, 32, 2016)
        sst = np.zeros((128, HEADS * SW), f32)
        for g in range(4):
            for h in range(HEADS):
                sst[32 * g:32 * g + 32, h * SW + 32 * g:h * SW + 32 * g + 2016] = flat[h]
    else:
        biasT = (pe[pidx] / scale).transpose(2, 1, 0)  # (H, j, i)
        sst = biasT.reshape(HEADS * NJT * 128, N)
    sst = sst.astype(ml_dtypes.bfloat16)

    common = dict(wqt=wqt, wkt=wkt, wvt=wvt, qkoff=qkoff, voffi=voffi,
                  wot=wot, ooff=ooff, sst=sst)
    in_maps = []
    for b in range(B):
        m = dict(common)
        m["x"] = np.ascontiguousarray(x[b].reshape(C, N))
        in_maps.append(m)
    return in_maps, structured


def run(inputs, trace=False, trace_cores=None):
    in_maps, structured = _fold_inputs(inputs)
    key = ("nc", structured)
    if key not in _PROGRAM_CACHE:
        _PROGRAM_CACHE[key] = build_program(structured)
    nc = _PROGRAM_CACHE[key]
    from concourse.bass_utils import run_bass_kernel_spmd
    res = run_bass_kernel_spmd(
        nc, in_maps, core_ids=list(range(B)), trace=trace, trace_cores=trace_cores
    )
    out = np.stack([res.results[b]["out"] for b in range(B)], axis=0)
    return out.reshape(B, C, F, F).astype(np.float32), res


def kernel(**inputs):
    out, _ = run(inputs, trace=False)
    return out


# revision 7
# speedup vs baseline: 1.4168x; 1.4168x over previous
"""Trainium2 Bass kernel for nn_Attention_66907000537586.

Module: x -> 1x1conv+BN (Q,K,V) -> 8-head attention with relative position
bias -> GELU -> 1x1conv+bias+BN.  Shapes: B=8, C=256, F=32 (n=1024 tokens),
H=8, DK=32, DV=64.

Sharding: pure data-parallel over batch (one batch element per NeuronCore,
8 cores), no collectives.  All BN/scale folding happens on host; the
relative-position bias is compressed host-side into its 63 distinct
32x32 Toeplitz blocks per head and injected into the dots PSUM on device
via identity matmuls (accumulating on top of the Q.K^T product).

Device dataflow per core (batch element):
  X (256,1024) --PE--> Q,K (256,1024 fp32, BN folded, scale folded into Q)
                 --PE--> V^T (1024, 8*(64+1) interleaved, bf16; per head 64 V
                         columns + a ones column that makes the attention
                         matmul also produce the softmax denominator)
  per head h, per j-tile (128 rows of K):
     dots^T[j,i] = K_h^T Q_h via matmul (K=32 contraction)  [PSUM fp32]
     + bias^T via identity-matmul copy from the compact bias table [accum]
     exp() on ScalarE PSUM->SBUF bf16 (softmax without max-subtraction --
       dots+bias is in [-8, 8], exp is safe in fp32/bf16)
     OU^T[d,i] += V^T_h(j-tile)^T @ exp-tile   [accumulated over j-tiles]
  OU rows 0..63 = unnormalized output, row 64 = sum_j exp = denominator S
  recip(S) on VectorE, broadcast across partitions via SBUF->SBUF DMA,
  normalize on VectorE, exact GELU on ScalarE -> G (512, 1024) fp32
  Y = Wo_folded @ G + bias per-partition  -> out (256, 1024) fp32
"""

import numpy as np
import ml_dtypes

HEADS, DK, DV, F = 8, 32, 64, 32
C = 256
N = F * F            # 1024 tokens
B = 8
EPS = 1e-5
IDK = HEADS * DK     # 256
IDV = HEADS * DV     # 512
VTW = HEADS * (DV + 1)   # 520: per head 64 V cols + 1 ones col (ones LAST)
SW = 2112            # per-head width of the shifted compact bias table
NJT = N // 128       # 8 j-tiles
NIT = N // 512       # 2 i-tiles

_PROGRAM_CACHE = {}


def _split_excess_waits(nc, mybir, limit=1):
    """The walrus build in this container rejects instructions carrying more
    than `limit` semaphore sync-waits.  Move the excess onto carrier NoOps
    inserted just before, on the same engine (same queue => same ordering)."""
    k = 0
    for fn in nc.m.functions:
        for bb in fn.blocks:
            out = []
            for inst in bb.instructions:
                si = inst.sync_info
                waits = list(si.on_wait) if si is not None else []
                if len(waits) > limit:
                    extra, keep = waits[:-limit], waits[-limit:]
                    for i in range(0, len(extra), limit):
                        nop = mybir.InstNoOp(name=f"waitsplit_{k}")
                        k += 1
                        nop.engine = inst.engine
                        nop.sync_info = mybir.SyncInfo(
                            on_wait=extra[i:i + limit], on_update=[])
                        out.append(nop)
                    si.on_wait = keep
                out.append(inst)
            bb.instructions = out


def build_program(structured=True):
    """Build the single-core Bass program (run SPMD on 8 cores)."""
    import concourse.bass as bass
    import concourse.mybir as mybir
    import concourse.tile as tile

    dt = mybir.dt
    PTC = tile.TileContext

    nc = bass.Bass("TRN2", target_bir_lowering=False, debug=False, num_devices=B)

    x = nc.dram_tensor("x", [C, N], dt.float32, kind="ExternalInput")
    wqt = nc.dram_tensor("wqt", [C, IDK], dt.float32, kind="ExternalInput")
    wkt = nc.dram_tensor("wkt", [C, IDK], dt.float32, kind="ExternalInput")
    wvt = nc.dram_tensor("wvt", [C, VTW], dt.float32, kind="ExternalInput")
    qkoff = nc.dram_tensor("qkoff", [128, 4], dt.float32, kind="ExternalInput")
    voffi = nc.dram_tensor("voffi", [1, VTW], dt.float32, kind="ExternalInput")
    wot = nc.dram_tensor("wot", [IDV, C], dt.float32, kind="ExternalInput")
    ooff = nc.dram_tensor("ooff", [128, 2], dt.float32, kind="ExternalInput")
    if structured:
        sst = nc.dram_tensor("sst", [128, HEADS * SW], dt.bfloat16, kind="ExternalInput")
    else:
        sst = nc.dram_tensor("sst", [HEADS * NJT * 128, N], dt.bfloat16, kind="ExternalInput")
    out = nc.dram_tensor("out", [C, N], dt.float32, kind="ExternalOutput")

    ident_np = np.eye(128, dtype=ml_dtypes.bfloat16)
    ident_dram = nc.inline_tensor(ident_np, name="ident128")

    with PTC(nc) as tc:
        with (
            tc.tile_pool(name="persist", bufs=1) as pp,
            tc.tile_pool(name="exps", bufs=3) as ep,
            tc.tile_pool(name="norm", bufs=2) as np_pool,
            tc.tile_pool(name="bias_stream", bufs=3) as bp,
            tc.tile_pool(name="dramscratch", bufs=2, space="DRAM") as dp,
            tc.tile_pool(name="psA", bufs=2, space="PSUM") as psA,
            tc.tile_pool(name="psB", bufs=2, space="PSUM") as psB,
        ):
            f32 = dt.float32
            bf16 = dt.bfloat16

            # ---- persistent SBUF tiles + input DMAs ----
            xsb = [pp.tile([128, N], f32, tag=f"xsb{k}", name=f"xsb{k}") for k in range(2)]
            for k in range(2):
                nc.sync.dma_start(out=xsb[k], in_=x.ap()[k * 128:(k + 1) * 128, :])
            wqtsb = [pp.tile([128, IDK], f32, tag=f"wqt{k}", name=f"wqt{k}") for k in range(2)]
            wktsb = [pp.tile([128, IDK], f32, tag=f"wkt{k}", name=f"wkt{k}") for k in range(2)]
            wvtsb = [pp.tile([128, VTW], f32, tag=f"wvt{k}", name=f"wvt{k}") for k in range(2)]
            for k in range(2):
                nc.sync.dma_start(out=wqtsb[k], in_=wqt.ap()[k * 128:(k + 1) * 128, :])
                nc.sync.dma_start(out=wktsb[k], in_=wkt.ap()[k * 128:(k + 1) * 128, :])
                nc.sync.dma_start(out=wvtsb[k], in_=wvt.ap()[k * 128:(k + 1) * 128, :])
            qkoffsb = pp.tile([128, 4], f32, tag="qkoff")
            nc.sync.dma_start(out=qkoffsb, in_=qkoff.ap())
            voffisb = pp.tile([1, VTW], f32, tag="voffi")
            nc.sync.dma_start(out=voffisb, in_=voffi.ap())
            wotsb = [pp.tile([128, C], f32, tag=f"wot{k}", name=f"wot{k}") for k in range(4)]
            for k in range(4):
                nc.sync.dma_start(out=wotsb[k], in_=wot.ap()[k * 128:(k + 1) * 128, :])
            ooffsb = pp.tile([128, 2], f32, tag="ooff")
            nc.sync.dma_start(out=ooffsb, in_=ooff.ap())
            identsb = pp.tile([128, 128], bf16, tag="ident")
            nc.sync.dma_start(out=identsb, in_=ident_dram.ap())
            if structured:
                sstsb = pp.tile([128, HEADS * SW], bf16, tag="sst")
                for h in range(HEADS):  # split DMA so head 0 arrives first
                    nc.sync.dma_start(
                        out=sstsb[:, h * SW:(h + 1) * SW],
                        in_=sst.ap()[:, h * SW:(h + 1) * SW],
                    )
            onesrow = pp.tile([1, N], f32, tag="onesrow")
            nc.vector.memset(onesrow, 1.0)

            # ---- projections ----
            qsb = [pp.tile([128, N], bf16, tag=f"qsb{m}", name=f"qsb{m}") for m in range(2)]
            ksb = [pp.tile([128, N], bf16, tag=f"ksb{m}", name=f"ksb{m}") for m in range(2)]
            vtsb = [pp.tile([128, VTW], bf16, tag=f"vtsb{j}", name=f"vtsb{j}") for j in range(NJT)]

            for m in range(2):  # Q then K, channel tile m
                for (wsb, dst, col) in ((wqtsb, qsb, m), (wktsb, ksb, 2 + m)):
                    ps = psA.tile([128, N], f32, tag="pA")
                    for nt in range(NIT):
                        for k in range(2):
                            nc.tensor.matmul(
                                ps[:, nt * 512:(nt + 1) * 512],
                                lhsT=wsb[k][:, m * 128:(m + 1) * 128],
                                rhs=xsb[k][:, nt * 512:(nt + 1) * 512],
                                start=(k == 0), stop=(k == 1),
                            )
                    nc.vector.tensor_scalar_add(dst[m], ps, qkoffsb[:, col:col + 1])

            for j in range(NJT):  # V^T j-tile
                ps = psA.tile([128, VTW], f32, tag="pA")
                for (lo, hi) in ((0, 512), (512, VTW)):
                    for k in range(2):
                        nc.tensor.matmul(
                            ps[:, lo:hi],
                            lhsT=xsb[k][:, j * 128:(j + 1) * 128],
                            rhs=wvtsb[k][:, lo:hi],
                            start=(k == 0), stop=False,
                        )
                    nc.tensor.matmul(  # + ones row x voffi  (BN offset + ones col)
                        ps[:, lo:hi],
                        lhsT=onesrow[0:1, j * 128:(j + 1) * 128],
                        rhs=voffisb[0:1, lo:hi],
                        start=False, stop=True,
                    )
                nc.vector.tensor_copy(vtsb[j], ps)  # fp32 -> bf16 cast

            # ---- attention per head ----
            gsb = [pp.tile([128, N], f32, tag=f"gsb{t}", name=f"gsb{t}") for t in range(4)]

            for h in range(HEADS):
                qt, rp = h // 4, 32 * (h % 4)
                ou = psB.tile([65, N], f32, tag="ou")
                for j in range(NJT):
                    dots = psA.tile([128, N], f32, tag="pA")
                    for it in range(NIT):
                        sl = slice(it * 512, (it + 1) * 512)
                        nc.tensor.matmul(
                            dots[:, sl],
                            lhsT=ksb[qt][rp:rp + 32, j * 128:(j + 1) * 128],
                            rhs=qsb[qt][rp:rp + 32, sl],
                            start=True, stop=False,
                            tile_position=(rp, 0),
                        )
                        if structured:
                            off = h * SW + (31 - 4 * j) * 32 + it * 512
                            rhs_bias = sstsb[:, off:off + 512]
                        else:
                            if it == 0:
                                btile = bp.tile([128, N], bf16, tag="bt")
                                base = (h * NJT + j) * 128
                                nc.sync.dma_start(
                                    out=btile, in_=sst.ap()[base:base + 128, :])
                            rhs_bias = btile[:, sl]
                        nc.tensor.matmul(
                            dots[:, sl], lhsT=identsb, rhs=rhs_bias,
                            start=False, stop=True,
                        )
                    et = ep.tile([128, N], bf16, tag="et")
                    nc.scalar.activation(et, dots, mybir.ActivationFunctionType.Exp)
                    for it in range(NIT):
                        sl = slice(it * 512, (it + 1) * 512)
                        nc.tensor.matmul(
                            ou[:, sl],
                            lhsT=vtsb[j][:, h * 65:h * 65 + 65],
                            rhs=et[:, sl],
                            start=(j == 0), stop=(j == NJT - 1),
                        )
                lnS = np_pool.tile([1, N], f32, tag="lnS")
                nc.scalar.activation(lnS, ou[64:65, :], mybir.ActivationFunctionType.Ln)
                recip = np_pool.tile([1, N], f32, tag="recip")
                nc.scalar.activation(recip, lnS, mybir.ActivationFunctionType.Exp,
                                     scale=-1.0)
                rd = dp.tile([1, N], f32, tag="rd", name="rd")
                nc.sync.dma_start(out=rd, in_=recip)
                rb = np_pool.tile([64, N], f32, tag="rb")
                nc.sync.dma_start(
                    out=rb,
                    in_=bass.AP(tensor=rd.tensor, offset=rd.offset, ap=[[0, 64], [1, N]]),
                )
                nc.vector.tensor_mul(
                    gsb[h // 2][64 * (h % 2):64 * (h % 2) + 64, :],
                    ou[0:64, :], rb)

            for t in range(4):  # batched exact GELU, one table-set switch
                nc.scalar.activation(gsb[t], gsb[t],
                                     mybir.ActivationFunctionType.Gelu)

            # ---- output conv ----
            for m in range(2):
                ps = psA.tile([128, N], f32, tag="pA")
                for nt in range(NIT):
                    sl = slice(nt * 512, (nt + 1) * 512)
                    for k in range(4):
                        nc.tensor.matmul(
                            ps[:, sl],
                            lhsT=wotsb[k][:, m * 128:(m + 1) * 128],
                            rhs=gsb[k][:, sl],
                            start=(k == 0), stop=(k == 3),
                        )
                ysb = np_pool.tile([128, N], f32, tag="ysb")
                nc.vector.tensor_scalar_add(ysb, ps, ooffsb[:, m:m + 1])
                nc.sync.dma_start(out=out.ap()[m * 128:(m + 1) * 128, :], in_=ysb)

    _split_excess_waits(nc, mybir)
    return nc


def _fold_inputs(inp):
    """Host-side BN/scale folding + compact bias table construction."""
    f32 = np.float32
    scale = DK ** -0.5
    x = np.asarray(inp["x"], f32)

    def bn_fold(w, gam, bet, mu, var, s=1.0):
        inv = np.asarray(gam, f32) / np.sqrt(np.asarray(var, f32) + EPS)
        wf = (np.asarray(w, f32) * inv[:, None] * s).T.copy()       # (C, O)
        off = (np.asarray(bet, f32) - np.asarray(mu, f32) * inv) * s  # (O,)
        return wf, off

    wqt, qoff = bn_fold(inp["wq"], inp["qgam"], inp["qbet"], inp["qmu"], inp["qvar"], scale)
    wkt, koff = bn_fold(inp["wk"], inp["kgam"], inp["kbet"], inp["kmu"], inp["kvar"])
    wvt0, voff = bn_fold(inp["wv"], inp["vgam"], inp["vbet"], inp["vmu"], inp["vvar"])

    wvt = np.zeros((C, VTW), f32)
    voffi = np.zeros((1, VTW), f32)
    for h in range(HEADS):
        wvt[:, 65 * h:65 * h + 64] = wvt0[:, 64 * h:64 * h + 64]
        voffi[0, 65 * h:65 * h + 64] = voff[64 * h:64 * h + 64]
        voffi[0, 65 * h + 64] = 1.0   # ones column -> softmax denominator

    oinv = np.asarray(inp["ogam"], f32) / np.sqrt(np.asarray(inp["ovar"], f32) + EPS)
    wot = (np.asarray(inp["wo"], f32) * oinv[:, None]).T.copy()      # (512, 256)
    ooff_v = (np.asarray(inp["bo"], f32) * oinv
              + np.asarray(inp["obet"], f32) - np.asarray(inp["omu"], f32) * oinv)

    qkoff = np.stack([qoff[:128], qoff[128:], koff[:128], koff[128:]], axis=1).copy()
    ooff = np.stack([ooff_v[:128], ooff_v[128:]], axis=1).copy()

    pe = np.asarray(inp["pos_emb"], f32)             # (1024, 8)
    pidx = np.asarray(inp["pos_indices"])            # (1024, 1024) int32

    r = np.arange(F)
    pos = np.stack(np.meshgrid(r, r, indexing="ij"), axis=-1).reshape(-1, 2)
    rel = np.abs(pos[:, None, :] - pos[None, :, :])
    expected = (rel[..., 0] * F + rel[..., 1]).astype(pidx.dtype)
    structured = bool(np.array_equal(pidx, expected))

    if structured:
        dd = np.arange(63)
        xi_ = np.maximum(dd - 31, 0)
        xj_ = np.maximum(31 - dd, 0)
        yy = np.arange(F)
        I = xi_[:, None] * F + yy[None, :]           # (63, yi)
        J = xj_[:, None] * F + yy[None, :]           # (63, yj)
        idx = pidx[I[:, None, :], J[:, :, None]]     # (dd, yj, yi)
        sb = pe[idx] / scale                         # (dd, yj, yi, H)
        flat = np.ascontiguousarray(sb.transpose(3, 1, 0, 2)).reshape(HEADS, 32, 2016)
        sst = np.zeros((128, HEADS * SW), f32)
        for g in range(4):
            for h in range(HEADS):
                sst[32 * g:32 * g + 32, h * SW + 32 * g:h * SW + 32 * g + 2016] = flat[h]
    else:
        biasT = (pe[pidx] / scale).transpose(2, 1, 0)  # (H, j, i)
        sst = biasT.reshape(HEADS * NJT * 128, N)
    sst = sst.astype(ml_dtypes.bfloat16)

    common = dict(wqt=wqt, wkt=wkt, wvt=wvt, qkoff=qkoff, voffi=voffi,
                  wot=wot, ooff=ooff, sst=sst)
    in_maps = []
    for b in range(B):
        m = dict(common)
        m["x"] = np.ascontiguousarray(x[b].reshape(C, N))
        in_maps.append(m)
    return in_maps, structured


def run(inputs, trace=False, trace_cores=None):
    in_maps, structured = _fold_inputs(inputs)
    key = ("nc", structured)
    if key not in _PROGRAM_CACHE:
        _PROGRAM_CACHE[key] = build_program(structured)
    nc = _PROGRAM_CACHE[key]
    from concourse.bass_utils import run_bass_kernel_spmd
    res = run_bass_kernel_spmd(
        nc, in_maps, core_ids=list(range(B)), trace=trace, trace_cores=trace_cores
    )
    out = np.stack([res.results[b]["out"] for b in range(B)], axis=0)
    return out.reshape(B, C, F, F).astype(np.float32), res


def kernel(**inputs):
    out, _ = run(inputs, trace=False)
    return out


# revision 9
# speedup vs baseline: 2.7652x; 1.9517x over previous
"""Trainium2 Bass kernel for nn_Attention_66907000537586.

Module: x -> 1x1conv+BN (Q,K,V) -> 8-head attention with relative position
bias -> exact GELU -> 1x1conv+bias+BN.  Shapes: B=8, C=256, F=32 (n=1024
tokens), H=8, DK=32, DV=64.

Sharding: pure data-parallel over batch (one batch element per NeuronCore,
8 cores), no collectives.  BN/scale folding happens on host.  The relative
position bias is block-Toeplitz (only 63 distinct 32x32 blocks per head);
the host builds a compact *shifted* table of exp(bias) so a single 2D SBUF
slice yields any (128 x 1024) transposed-bias tile, and softmax uses
exp(dots+bias) = exp(dots) * exp(bias) with the multiply on VectorE.

Device dataflow per core (batch element b):
  X (256,1024) fp32 -> cast bf16
  Q,K = folded 1x1conv (bf16 matmul, PSUM fp32, per-channel BN offset added
        during evacuation, attention scale folded into Q) -> bf16
  V^T (1024 x 8*(64+1), bf16): per head 64 V columns + a ones column so the
        attention matmul also produces the softmax denominator S.
  per head pair (row-group-disjoint => their dots matmuls overlap on PE),
  per j-tile (128 rows of K):
     dotsT[j,i] = K_h^T Q_h   (K=32 contraction, tile_position row packing)
     et  = exp(dotsT) on ScalarE (PSUM->SBUF bf16; no max-subtraction needed,
           dots is O(+-6))
     et2 = et * expbias-slice on VectorE (bf16 2x mode)
     OU^T[d,i] += V^T_h(j-tile)^T @ et2    (accumulated over j-tiles)
  OU rows 0..63 = unnormalized output, row 64 = S
  1/S = exp(-ln(S)) on ScalarE (stays in the natural_log_exp table set),
  broadcast across partitions via a DRAM bounce DMA, normalize on VectorE
  -> G bf16; batched exact GELU at the end (one gelu table-set switch)
  Y = Wo_folded @ G (bf16) + per-channel offset -> out (256,1024) fp32
"""

import numpy as np
import ml_dtypes

HEADS, DK, DV, F = 8, 32, 64, 32
C = 256
N = F * F            # 1024 tokens
B = 8
EPS = 1e-5
IDK = HEADS * DK     # 256
IDV = HEADS * DV     # 512
VTW = HEADS * (DV + 1)   # 520
SW = 2112            # per-head width of the shifted compact bias table
NJT = N // 128       # 8 j-tiles
NIT = N // 512       # 2 i-tiles

_PROGRAM_CACHE = {}


def _split_excess_waits(nc, mybir, limit=1):
    """The walrus build in this container rejects instructions carrying more
    than `limit` semaphore sync-waits.  Move the excess onto carrier NoOps
    inserted just before, on the same engine (same queue => same ordering)."""
    k = 0
    for fn in nc.m.functions:
        for bb in fn.blocks:
            out = []
            for inst in bb.instructions:
                si = inst.sync_info
                waits = list(si.on_wait) if si is not None else []
                if len(waits) > limit:
                    extra, keep = waits[:-limit], waits[-limit:]
                    for i in range(0, len(extra), limit):
                        nop = mybir.InstNoOp(name=f"waitsplit_{k}")
                        k += 1
                        nop.engine = inst.engine
                        nop.sync_info = mybir.SyncInfo(
                            on_wait=extra[i:i + limit], on_update=[])
                        out.append(nop)
                    si.on_wait = keep
                out.append(inst)
            bb.instructions = out


def build_program(structured=True):
    """Build the single-core Bass program (run SPMD on 8 cores)."""
    import concourse.bass as bass
    import concourse.mybir as mybir
    import concourse.tile as tile

    dt = mybir.dt
    nc = bass.Bass("TRN2", target_bir_lowering=False, debug=False, num_devices=B)

    f32, bf16 = dt.float32, dt.bfloat16

    x = nc.dram_tensor("x", [C, N], f32, kind="ExternalInput")
    wqt = nc.dram_tensor("wqt", [C, IDK], bf16, kind="ExternalInput")
    wkt = nc.dram_tensor("wkt", [C, IDK], bf16, kind="ExternalInput")
    wvt = nc.dram_tensor("wvt", [C, VTW], bf16, kind="ExternalInput")
    qkoff = nc.dram_tensor("qkoff", [128, 4], f32, kind="ExternalInput")
    voffi = nc.dram_tensor("voffi", [1, VTW], bf16, kind="ExternalInput")
    wot = nc.dram_tensor("wot", [IDV, C], bf16, kind="ExternalInput")
    ooff = nc.dram_tensor("ooff", [128, 2], f32, kind="ExternalInput")
    if structured:
        sst = nc.dram_tensor("sst", [128, HEADS * SW], bf16, kind="ExternalInput")
    else:
        sst = nc.dram_tensor("sst", [HEADS * NJT * 128, N], bf16, kind="ExternalInput")
    out = nc.dram_tensor("out", [C, N], f32, kind="ExternalOutput")

    ident_dram = None
    if not structured:
        ident_np = np.eye(128, dtype=ml_dtypes.bfloat16)
        ident_dram = nc.inline_tensor(ident_np, name="ident128")

    with tile.TileContext(nc) as tc:
        with (
            tc.tile_pool(name="persist", bufs=1) as pp,
            tc.tile_pool(name="exps", bufs=2) as ep,
            tc.tile_pool(name="exps2", bufs=3) as e2p,
            tc.tile_pool(name="norm", bufs=2) as np_pool,
            tc.tile_pool(name="bias_stream", bufs=3) as bp,
            tc.tile_pool(name="dramscratch", bufs=2, space="DRAM") as dp,
            tc.tile_pool(name="psA", bufs=2, space="PSUM") as psA,
            tc.tile_pool(name="psB", bufs=2, space="PSUM") as psB,
        ):
            # ---- persistent SBUF tiles + input DMAs ----
            xsb = [pp.tile([128, N], f32, tag=f"xsb{k}", name=f"xsb{k}")
                   for k in range(2)]
            for k in range(2):
                nc.sync.dma_start(out=xsb[k], in_=x.ap()[k * 128:(k + 1) * 128, :])
            wqtsb = [pp.tile([128, IDK], bf16, tag=f"wqt{k}", name=f"wqt{k}")
                     for k in range(2)]
            wktsb = [pp.tile([128, IDK], bf16, tag=f"wkt{k}", name=f"wkt{k}")
                     for k in range(2)]
            wvtsb = [pp.tile([128, VTW], bf16, tag=f"wvt{k}", name=f"wvt{k}")
                     for k in range(2)]
            for k in range(2):
                nc.sync.dma_start(out=wqtsb[k], in_=wqt.ap()[k * 128:(k + 1) * 128, :])
                nc.sync.dma_start(out=wktsb[k], in_=wkt.ap()[k * 128:(k + 1) * 128, :])
                nc.sync.dma_start(out=wvtsb[k], in_=wvt.ap()[k * 128:(k + 1) * 128, :])
            qkoffsb = pp.tile([128, 4], f32, tag="qkoff")
            nc.sync.dma_start(out=qkoffsb, in_=qkoff.ap())
            voffisb = pp.tile([1, VTW], bf16, tag="voffi")
            nc.sync.dma_start(out=voffisb, in_=voffi.ap())
            wotsb = [pp.tile([128, C], bf16, tag=f"wot{k}", name=f"wot{k}")
                     for k in range(4)]
            for k in range(4):
                nc.sync.dma_start(out=wotsb[k], in_=wot.ap()[k * 128:(k + 1) * 128, :])
            ooffsb = pp.tile([128, 2], f32, tag="ooff")
            nc.sync.dma_start(out=ooffsb, in_=ooff.ap())
            identsb = None
            if not structured:
                identsb = pp.tile([128, 128], bf16, tag="ident")
                nc.sync.dma_start(out=identsb, in_=ident_dram.ap())
            sstsb = None
            if structured:
                sstsb = pp.tile([128, HEADS * SW], bf16, tag="sst")
                for h in range(HEADS):  # split DMA so head 0 arrives first
                    nc.sync.dma_start(
                        out=sstsb[:, h * SW:(h + 1) * SW],
                        in_=sst.ap()[:, h * SW:(h + 1) * SW],
                    )
            onesrow = pp.tile([1, N], bf16, tag="onesrow")
            nc.vector.memset(onesrow, 1.0)

            # ---- projections (bf16) ----
            xbf = [pp.tile([128, N], bf16, tag=f"xbf{k}", name=f"xbf{k}")
                   for k in range(2)]
            for k in range(2):
                nc.vector.tensor_copy(xbf[k], xsb[k])
            qsb = [pp.tile([128, N], bf16, tag=f"qsb{m}", name=f"qsb{m}")
                   for m in range(2)]
            ksb = [pp.tile([128, N], bf16, tag=f"ksb{m}", name=f"ksb{m}")
                   for m in range(2)]
            vtsb = [pp.tile([128, VTW], bf16, tag=f"vtsb{j}", name=f"vtsb{j}")
                    for j in range(NJT)]

            for m in range(2):  # Q then K, channel tile m
                for (wsb, dst, col) in ((wqtsb, qsb, m), (wktsb, ksb, 2 + m)):
                    ps = psA.tile([128, N], f32, tag="pA", name="ps")
                    for nt in range(NIT):
                        for k in range(2):
                            nc.tensor.matmul(
                                ps[:, nt * 512:(nt + 1) * 512],
                                lhsT=wsb[k][:, m * 128:(m + 1) * 128],
                                rhs=xbf[k][:, nt * 512:(nt + 1) * 512],
                                start=(k == 0), stop=(k == 1),
                            )
                    nc.vector.tensor_scalar_add(dst[m], ps, qkoffsb[:, col:col + 1])

            for j in range(NJT):  # V^T j-tile
                ps = psA.tile([128, VTW], f32, tag="pA", name="ps")
                for (lo, hi) in ((0, 512), (512, VTW)):
                    for k in range(2):
                        nc.tensor.matmul(
                            ps[:, lo:hi],
                            lhsT=xbf[k][:, j * 128:(j + 1) * 128],
                            rhs=wvtsb[k][:, lo:hi],
                            start=(k == 0), stop=False,
                        )
                    nc.tensor.matmul(  # + ones row x voffi (BN offset + ones col)
                        ps[:, lo:hi],
                        lhsT=onesrow[0:1, j * 128:(j + 1) * 128],
                        rhs=voffisb[0:1, lo:hi],
                        start=False, stop=True,
                    )
                nc.vector.tensor_copy(vtsb[j], ps)  # fp32 -> bf16 cast

            # ---- attention, head pairs (disjoint PE row groups) ----
            gsb = [pp.tile([128, N], bf16, tag=f"gsb{t}", name=f"gsb{t}")
                   for t in range(4)]

            for hp in range(4):
                pair = (2 * hp, 2 * hp + 1)
                ous = {}
                for h in pair:
                    ous[h] = psB.tile([65, N], f32, tag="ou", name=f"ou{h}")
                for j in range(NJT):
                    dts = {}
                    for h in pair:
                        qt, rp = h // 4, 32 * (h % 4)
                        dots = psA.tile([128, N], f32, tag="pA", name=f"dots{h}")
                        dts[h] = dots
                        for it in range(NIT):
                            sl = slice(it * 512, (it + 1) * 512)
                            nc.tensor.matmul(
                                dots[:, sl],
                                lhsT=ksb[qt][rp:rp + 32, j * 128:(j + 1) * 128],
                                rhs=qsb[qt][rp:rp + 32, sl],
                                start=True, stop=structured,
                                tile_position=(rp, 0),
                            )
                            if not structured:
                                if it == 0:
                                    btile = bp.tile([128, N], bf16, tag="bt",
                                                    name="bt")
                                    base = (h * NJT + j) * 128
                                    nc.sync.dma_start(
                                        out=btile, in_=sst.ap()[base:base + 128, :])
                                nc.tensor.matmul(
                                    dots[:, sl], lhsT=identsb, rhs=btile[:, sl],
                                    start=False, stop=True,
                                )
                    for h in pair:
                        et = ep.tile([128, N], bf16, tag="et", name="et")
                        nc.scalar.activation(et, dts[h],
                                             mybir.ActivationFunctionType.Exp)
                        if structured:
                            off = h * SW + (31 - 4 * j) * 32
                            et2 = e2p.tile([128, N], bf16, tag="et2", name="et2")
                            nc.vector.tensor_mul(et2, et, sstsb[:, off:off + N])
                        else:
                            et2 = et
                        for it in range(NIT):
                            sl = slice(it * 512, (it + 1) * 512)
                            nc.tensor.matmul(
                                ous[h][:, sl],
                                lhsT=vtsb[j][:, h * 65:h * 65 + 65],
                                rhs=et2[:, sl],
                                start=(j == 0), stop=(j == NJT - 1),
                            )
                for h in pair:
                    ou = ous[h]
                    lnS = np_pool.tile([1, N], f32, tag="lnS", name="lnS")
                    nc.scalar.activation(lnS, ou[64:65, :],
                                         mybir.ActivationFunctionType.Ln)
                    recip = np_pool.tile([1, N], f32, tag="recip", name="recip")
                    nc.scalar.activation(recip, lnS,
                                         mybir.ActivationFunctionType.Exp,
                                         scale=-1.0)
                    rd = dp.tile([1, N], f32, tag="rd", name="rd")
                    nc.sync.dma_start(out=rd, in_=recip)
                    rb = np_pool.tile([64, N], f32, tag="rb", name="rb")
                    nc.sync.dma_start(
                        out=rb,
                        in_=bass.AP(tensor=rd.tensor, offset=rd.offset,
                                    ap=[[0, 64], [1, N]]),
                    )
                    nc.vector.tensor_mul(
                        gsb[h // 2][64 * (h % 2):64 * (h % 2) + 64, :],
                        ou[0:64, :], rb)

            for t in range(4):  # batched exact GELU, one table-set switch
                nc.scalar.activation(gsb[t], gsb[t],
                                     mybir.ActivationFunctionType.Gelu)

            # ---- output conv (bf16) ----
            for m in range(2):
                ps = psA.tile([128, N], f32, tag="pA", name="ps")
                for nt in range(NIT):
                    sl = slice(nt * 512, (nt + 1) * 512)
                    for k in range(4):
                        nc.tensor.matmul(
                            ps[:, sl],
                            lhsT=wotsb[k][:, m * 128:(m + 1) * 128],
                            rhs=gsb[k][:, sl],
                            start=(k == 0), stop=(k == 3),
                        )
                ysb = np_pool.tile([128, N], f32, tag="ysb", name="ysb")
                nc.vector.tensor_scalar_add(ysb, ps, ooffsb[:, m:m + 1])
                nc.sync.dma_start(out=out.ap()[m * 128:(m + 1) * 128, :], in_=ysb)

    _split_excess_waits(nc, mybir)
    return nc


def _fold_inputs(inp):
    """Host-side BN/scale folding + compact bias table construction."""
    f32 = np.float32
    bfc = ml_dtypes.bfloat16
    scale = DK ** -0.5
    x = np.asarray(inp["x"], f32)

    def bn_fold(w, gam, bet, mu, var, s=1.0):
        inv = np.asarray(gam, f32) / np.sqrt(np.asarray(var, f32) + EPS)
        wf = (np.asarray(w, f32) * inv[:, None] * s).T.copy()        # (C, O)
        off = (np.asarray(bet, f32) - np.asarray(mu, f32) * inv) * s  # (O,)
        return wf, off

    wqt, qoff = bn_fold(inp["wq"], inp["qgam"], inp["qbet"], inp["qmu"],
                        inp["qvar"], scale)
    wkt, koff = bn_fold(inp["wk"], inp["kgam"], inp["kbet"], inp["kmu"],
                        inp["kvar"])
    wvt0, voff = bn_fold(inp["wv"], inp["vgam"], inp["vbet"], inp["vmu"],
                         inp["vvar"])

    wvt = np.zeros((C, VTW), f32)
    voffi = np.zeros((1, VTW), f32)
    for h in range(HEADS):
        wvt[:, 65 * h:65 * h + 64] = wvt0[:, 64 * h:64 * h + 64]
        voffi[0, 65 * h:65 * h + 64] = voff[64 * h:64 * h + 64]
        voffi[0, 65 * h + 64] = 1.0   # ones column -> softmax denominator

    oinv = np.asarray(inp["ogam"], f32) / np.sqrt(np.asarray(inp["ovar"], f32) + EPS)
    wot = (np.asarray(inp["wo"], f32) * oinv[:, None]).T.copy()       # (512, 256)
    ooff_v = (np.asarray(inp["bo"], f32) * oinv
              + np.asarray(inp["obet"], f32) - np.asarray(inp["omu"], f32) * oinv)

    qkoff = np.stack([qoff[:128], qoff[128:], koff[:128], koff[128:]], axis=1).copy()
    ooff = np.stack([ooff_v[:128], ooff_v[128:]], axis=1).copy()

    pe = np.asarray(inp["pos_emb"], f32)             # (1024, 8)
    pidx = np.asarray(inp["pos_indices"])            # (1024, 1024) int32

    r = np.arange(F)
    pos = np.stack(np.meshgrid(r, r, indexing="ij"), axis=-1).reshape(-1, 2)
    rel = np.abs(pos[:, None, :] - pos[None, :, :])
    expected = (rel[..., 0] * F + rel[..., 1]).astype(pidx.dtype)
    structured = bool(np.array_equal(pidx, expected))

    if structured:
        dd = np.arange(63)
        xi_ = np.maximum(dd - 31, 0)
        xj_ = np.maximum(31 - dd, 0)
        yy = np.arange(F)
        I = xi_[:, None] * F + yy[None, :]           # (63, yi)
        J = xj_[:, None] * F + yy[None, :]           # (63, yj)
        idx = pidx[I[:, None, :], J[:, :, None]]     # (dd, yj, yi)
        sb = pe[idx] / scale                         # (dd, yj, yi, H)
        flat = np.ascontiguousarray(sb.transpose(3, 1, 0, 2)).reshape(HEADS, 32, 2016)
        eflat = np.exp(flat)   # exp(dots+bias) = exp(dots) * exp(bias)
        sstv = np.zeros((128, HEADS * SW), f32)
        for g in range(4):
            for h in range(HEADS):
                sstv[32 * g:32 * g + 32,
                     h * SW + 32 * g:h * SW + 32 * g + 2016] = eflat[h]
    else:
        biasT = (pe[pidx] / scale).transpose(2, 1, 0)  # (H, j, i)
        sstv = biasT.reshape(HEADS * NJT * 128, N)
    sstv = sstv.astype(bfc)

    common = dict(wqt=wqt.astype(bfc), wkt=wkt.astype(bfc), wvt=wvt.astype(bfc),
                  qkoff=qkoff, voffi=voffi.astype(bfc),
                  wot=wot.astype(bfc), ooff=ooff, sst=sstv)
    in_maps = []
    for b in range(B):
        m = dict(common)
        m["x"] = np.ascontiguousarray(x[b].reshape(C, N))
        in_maps.append(m)
    return in_maps, structured


def run(inputs, trace=False, trace_cores=None):
    in_maps, structured = _fold_inputs(inputs)
    key = ("nc", structured)
    if key not in _PROGRAM_CACHE:
        _PROGRAM_CACHE[key] = build_program(structured)
    nc = _PROGRAM_CACHE[key]
    from concourse.bass_utils import run_bass_kernel_spmd
    res = run_bass_kernel_spmd(
        nc, in_maps, core_ids=list(range(B)), trace=trace, trace_cores=trace_cores
    )
    out = np.stack([res.results[b]["out"] for b in range(B)], axis=0)
    return out.reshape(B, C, F, F).astype(np.float32), res


def kernel(**inputs):
    out, _ = run(inputs, trace=False)
    return out
